# revision 31
# baseline (speedup 1.0000x reference)
"""Peephole Conv-LSTM (T=8,B=8,C=64,H=W=96,L=2,K=3) on 8 Trainium2 cores.

Strategy
--------
Data-parallel over batch: core b processes batch item b through the whole
T x L recurrence (no collectives).

Conv-as-matmul: images live in SBUF as [C=64 partitions, 98*98 flat padded
pixels].  A 3x3 SAME conv is 9 shifted matmuls accumulating in PSUM, where
the shift is just a flat AP offset into the padded image.

Partition packing (K=128 contraction, M=128 outputs per matmul):
  * xh combo  [x_pad | h_pad]      -> conv(x,Wx*) + conv(h,Wh*) fuse into one
    K=128 matmul per tap; M packs two gates per pass.
  * cc combo  [c | c<<1]  and  cc98 combo [c | c<<98] -> peephole convs pair
    taps (dy,0)+(dy,1) and (0,2)+(1,2) in K; only (2,2) is a singleton with
    zeroed bottom rows.  5 matmuls per 9-tap peephole conv instead of 9.

Per (layer, t): phase A computes i,f (psum [i|f]) and o_partial,ct
(psum [o|ct]) over 19 flat pixel chunks of <=496; c_new = ct*i + c*f written
into ping-pong cc/cc98 buffers + dup-shift copies.  Phase B (pipelined TWO
chunks behind A so the dup DMA latency hides under A's matmuls) accumulates
conv(c_new,Whc) directly onto the o-half of the SAME psum bank, then
sigmoid/tanh/mul produce h_new.  The reference's quirk (o-gate reuses
Whc + bc on the new cell state) is baked into phase B weights/biases.

x inputs double-buffer through the xh ping-pong pair: the load for step
s+1 is issued at the start of step s into the tile A(s) is not reading.
"""

import os

import numpy as np

import concourse.bass as bass
import concourse.mybir as mybir
from concourse import bacc
from concourse.bass_utils import run_bass_kernel_spmd
from concourse.tile import TileContext

F32 = mybir.dt.float32
F16 = mybir.dt.float16
AF = mybir.ActivationFunctionType

T, B, C, H, W, L, KS = 8, 8, 64, 96, 96, 2, 3
Hp = Wp = 98
NPIX = Hp * Wp              # 9604
ALLOC = NPIX + 1            # slack elem so shifted dup reads stay in bounds
W0 = Wp + 1                 # flat index of output pixel (0,0) in padded coords
NWIN = 96 * Wp + 96 - W0 + 1  # 9406 flat positions spanning all output pixels
CH = 496
CHUNKS = [(W0 + i * CH, min(CH, NWIN - i * CH)) for i in range((NWIN + CH - 1) // CH)]

# peephole 5-block structure: (tile_kind, tap_of_rows_0_64)
#   tile_kind 0 -> cc   [c | c<<1]
#   tile_kind 1 -> cc98 [c | c<<98]
# taps as (dy,dx); row-half-1 tap = half-0 tap shifted +1 (cc) or +98 (cc98)
PEEP_BLOCKS = [(0, (0, 0)), (0, (1, 0)), (0, (2, 0)), (1, (0, 2)), (0, (2, 2))]


def _tap_shift(dy, dx):
    return (dy - 1) * Wp + (dx - 1)


# profiling side-channel for test.py
LAST_EXEC_NS = None
LAST_RES = None


def _build(t_steps: int, n_layers: int, reps: int = 1) -> bass.Bass:
    nc = bacc.Bacc("TRN2", target_bir_lowering=False, debug=False,
                   enable_asserts=False, num_devices=8)

    n_steps = t_steps * n_layers
    xp = nc.declare_dram_parameter("xp", [t_steps, C, ALLOC], F16, isOutput=False)
    wd = {}
    for l in range(n_layers):
        wd[f"aif{l}"] = nc.declare_dram_parameter(f"aif{l}", [128, 9 * 128], F16, isOutput=False)
        wd[f"cif{l}"] = nc.declare_dram_parameter(f"cif{l}", [128, 5 * 128], F16, isOutput=False)
        wd[f"acto{l}"] = nc.declare_dram_parameter(f"acto{l}", [128, 9 * 128], F16, isOutput=False)
        wd[f"whc{l}"] = nc.declare_dram_parameter(f"whc{l}", [128, 5 * 64], F16, isOutput=False)
        wd[f"bif{l}"] = nc.declare_dram_parameter(f"bif{l}", [128, 1], F32, isOutput=False)
        wd[f"bc{l}"] = nc.declare_dram_parameter(f"bc{l}", [64, 1], F32, isOutput=False)
        wd[f"bobc{l}"] = nc.declare_dram_parameter(f"bobc{l}", [64, 1], F32, isOutput=False)
    hs = nc.declare_dram_parameter("hs", [t_steps, C, ALLOC], F16, isOutput=True)
    cs = nc.declare_dram_parameter("cs", [t_steps, C, ALLOC], F16, isOutput=True)
    hscr = nc.dram_tensor("hscr", [t_steps, C, ALLOC], F16) if n_layers > 1 else None

    with TileContext(nc) as tc:
        with (
            tc.tile_pool(name="big", bufs=1) as bigp,
            tc.tile_pool(name="wp", bufs=1) as wp,
            tc.tile_pool(name="gp", bufs=1) as gp,
            tc.tile_pool(name="pif", bufs=3, space="PSUM") as pool_pif,
            tc.tile_pool(name="pcto", bufs=5, space="PSUM") as pool_pcto,
        ):
            xh = [bigp.tile([128, ALLOC], F16, name="xh0"),
                  bigp.tile([128, ALLOC], F16, name="xh1")]
            cc = [bigp.tile([128, ALLOC], F16, name="cc0"),
                  bigp.tile([128, ALLOC], F16, name="cc1")]
            cc98 = [bigp.tile([128, ALLOC], F16, name="cc98_0"),
                    bigp.tile([128, ALLOC], F16, name="cc98_1")]

            # x0 load first (before weights/memsets) so phase A of step 0
            # isn't queued behind them; striped so chunk 0 starts early.
            # The x half of xh[0] is fully overwritten by this load, so the
            # zero-init below only touches the h halves.
            XQ = ALLOC // 8
            for q in range(8):
                hi = ALLOC if q == 7 else (q + 1) * XQ
                nc.sync.dma_start(xh[0][0:64, q * XQ:hi], xp[0, :, q * XQ:hi])

            wsb = {}
            for l in range(n_layers):
                for nm, cols in (("aif", 9 * 128), ("cif", 5 * 128),
                                 ("acto", 9 * 128), ("whc", 5 * 64)):
                    tl = wp.tile([128, cols], F16, name=f"{nm}sb{l}")
                    if l == 0 and nm in ("aif", "acto"):
                        # per-tap stripes: step 0's first matmuls start after
                        # one 32KB stripe instead of the full 294KB block
                        for j in range(9):
                            nc.sync.dma_start(tl[:, j * 128:(j + 1) * 128],
                                              wd[f"{nm}{l}"][:, j * 128:(j + 1) * 128])
                    else:
                        nc.sync.dma_start(tl[:], wd[f"{nm}{l}"][:])
                    wsb[f"{nm}{l}"] = tl
                for nm, p in (("bif", 128), ("bc", 64), ("bobc", 64)):
                    tl = wp.tile([p, 1], F32, name=f"{nm}sb{l}")
                    nc.sync.dma_start(tl[:], wd[f"{nm}{l}"][:])
                    wsb[f"{nm}{l}"] = tl

            # Init-zero ONLY the leading/trailing edge columns: every interior
            # byte of these tiles is (ring-zeroed and) written before it is
            # read, so full-tile memsets would just stall the Vector FIFO for
            # ~20us at step 0.  Edges per tile cover the union of what the
            # plain half and its shifted dup half can read uninitialized.
            for tl, lo, hi in ((xh[0], W0, NPIX - 99), (xh[1], W0, NPIX - 99),
                               (cc[0], W0, NPIX - 100), (cc[1], W0, NPIX - 100),
                               (cc98[0], W0, NWIN - 1), (cc98[1], W0, NWIN - 1)):
                nc.vector.memset(tl[:, 0:lo], 0.0)
                nc.vector.memset(tl[:, hi:ALLOC], 0.0)

            # PE warmup burst: ~20 dependency-light matmuls so the HAM clock
            # gate reaches 8/8 before step 0's real matmuls (results unused)
            for wi in range(20):
                p_w = pool_pif.tile([128, CH], F32, tag="pif")
                nc.tensor.matmul(p_w[:, :CH],
                                 wsb["aif0"][:, 0:128],
                                 wsb["aif0"][:, 128:128 + CH],
                                 start=True, stop=True)

            g_if = gp.tile([128, 2 * CH], F16, name="g_if")
            g_osum = gp.tile([64, 2 * CH], F32, name="g_osum")
            g_ct = gp.tile([64, 2 * CH], F16, name="g_ct")
            g_tmp = gp.tile([64, 2 * CH], F16, name="g_tmp")
            g_th = gp.tile([64, 2 * CH], F16, name="g_th")
            g_osb = gp.tile([64, 2 * CH], F16, name="g_osb")

            def src_of(step):
                l, t = divmod(step, t_steps)
                return (xp if l == 0 else hscr), t

            def ring_zero(tile, p0, w0, n):
                # zero pad-ring cols 0/97 garbage inside flat [w0, w0+n);
                # on GpSimd — it's otherwise idle and this keeps DVE free
                for rem in (0, Wp - 1):
                    first = ((w0 - rem + Wp - 1) // Wp) * Wp + rem
                    if first >= w0 + n:
                        continue
                    cnt = (w0 + n - 1 - first) // Wp + 1
                    v = tile[p0:p0 + 64, first:first + Wp * cnt]
                    v = v.rearrange("p (r w) -> p r w", w=Wp)
                    nc.gpsimd.memset(v[:, :, 0:1], 0.0)

            step = 0
            for _rep in range(reps):
             for l in range(n_layers):
              for t in range(t_steps):
                s = step
                cur_x, nxt_x = xh[s % 2], xh[(s + 1) % 2]
                cur_c, nxt_c = cc[s % 2], cc[(s + 1) % 2]
                cur_c98, nxt_c98 = cc98[s % 2], cc98[(s + 1) % 2]
                aifW, cifW = wsb[f"aif{l}"], wsb[f"cif{l}"]
                actoW, whcW = wsb[f"acto{l}"], wsb[f"whc{l}"]

                first = (t == 0)  # h_0 = c_0 = 0: K=64 matmuls on the x half
                # only, no peephole matmuls, no c*f term — so no state resets
                # are needed at the layer boundary.

                # prefetch next step's x into the other xh buffer
                if s + 1 < n_steps:
                    srcn, tn = src_of(s + 1)
                    nc.sync.dma_start(nxt_x[0:64, :], srcn[tn, :, :])

                pcto_tiles = {}

                def phase_a(k, cur_x=cur_x, cur_c=cur_c, cur_c98=cur_c98,
                            nxt_c=nxt_c, nxt_c98=nxt_c98, aifW=aifW,
                            cifW=cifW, actoW=actoW, l=l, first=first,
                            pcto_tiles=pcto_tiles):
                    w0, n = CHUNKS[k]
                    kp = 64 if first else 128  # t=0: contract x half only
                    p_if = pool_pif.tile([128, CH], F32, tag="pif")
                    p_ct = pool_pcto.tile([128, CH], F32, tag="pcto")
                    pcto_tiles[k] = p_ct
                    for j in range(9):
                        dy, dx = divmod(j, 3)
                        sft = w0 + _tap_shift(dy, dx)
                        nc.tensor.matmul(
                            p_if[:, :n],
                            aifW[0:kp, j * 128:(j + 1) * 128],
                            cur_x[0:kp, sft:sft + n],
                            start=(j == 0), stop=(first and j == 8))
                    if not first:
                        for b, (kind, (dy, dx)) in enumerate(PEEP_BLOCKS):
                            tile = cur_c if kind == 0 else cur_c98
                            sft = w0 + _tap_shift(dy, dx)
                            nc.tensor.matmul(
                                p_if[:, :n],
                                cifW[:, b * 128:(b + 1) * 128],
                                tile[:, sft:sft + n],
                                start=False, stop=(b == 4))
                    for j in range(9):
                        dy, dx = divmod(j, 3)
                        sft = w0 + _tap_shift(dy, dx)
                        # stop=True closes the sim's accumulation-group so the
                        # ct half can be read now; phase B reopens with
                        # skip_group_check and accumulates onto the o half.
                        nc.tensor.matmul(
                            p_ct[:, :n],
                            actoW[0:kp, j * 128:(j + 1) * 128],
                            cur_x[0:kp, sft:sft + n],
                            start=(j == 0), stop=(j == 8))

                    sl = slice((k % 2) * CH, (k % 2) * CH + n)
                    wsl = slice(w0, w0 + n)
                    nc.scalar.activation(g_if[:, sl], p_if[:, :n], AF.Sigmoid,
                                         bias=wsb[f"bif{l}"][:])
                    nc.scalar.activation(g_ct[:, sl], p_ct[64:128, :n], AF.Tanh,
                                         bias=wsb[f"bc{l}"][:])
                    if first:
                        # c_new = ct*i
                        nc.vector.tensor_mul(nxt_c[0:64, wsl], g_ct[:, sl],
                                             g_if[0:64, sl])
                    else:
                        # c_new = c*f + ct*i ; read c via the +1-shift dup half
                        # so both TensorTensor inputs share start partition 64
                        nc.vector.tensor_mul(nxt_c[0:64, wsl],
                                             cur_c[64:128, w0 - 1:w0 - 1 + n],
                                             g_if[64:128, sl])
                        nc.vector.tensor_mul(g_tmp[:, sl], g_ct[:, sl],
                                             g_if[0:64, sl])
                        nc.vector.tensor_add(nxt_c[0:64, wsl],
                                             nxt_c[0:64, wsl],
                                             g_tmp[:, sl])
                    # ring-zero this chunk of c_new, then its dup stripes
                    ring_zero(nxt_c, 0, w0, n)
                    nc.sync.dma_start(nxt_c[64:128, w0 - 1:w0 - 1 + n],
                                      nxt_c[0:64, w0:w0 + n])
                    # same-partition dup on GpSimd: relieves the Sync DMA
                    # dispatcher (near saturation) at ~0.4us GpSimd cost
                    nc.gpsimd.tensor_copy(nxt_c98[0:64, w0:w0 + n],
                                          nxt_c[0:64, w0:w0 + n])
                    nc.sync.dma_start(nxt_c98[64:128, w0 - 98:w0 - 98 + n],
                                      nxt_c[0:64, w0:w0 + n])

                def phase_b(k, nxt_x=nxt_x, nxt_c=nxt_c, nxt_c98=nxt_c98,
                            whcW=whcW, l=l, pcto_tiles=pcto_tiles):
                    w0, n = CHUNKS[k]
                    p_ct = pcto_tiles[k]
                    # col-split: blocks 0-2 accumulate onto the o half
                    # (partitions 0:64, col group 0); blocks 3-4 overwrite the
                    # consumed ct half (col group 1) and run concurrently on
                    # the other half of the PE array.  Interleave issue order
                    # so the hardware overlaps adjacent different-col MMs.
                    order = [0, 3, 1, 4, 2]
                    for b in order:
                        kind, (dy, dx) = PEEP_BLOCKS[b]
                        tile = nxt_c if kind == 0 else nxt_c98
                        sft = w0 + _tap_shift(dy, dx)
                        if b < 3:
                            out, st, sp = p_ct[0:64, :n], False, (b == 2)
                        else:
                            out, st, sp = p_ct[64:128, :n], (b == 3), (b == 4)
                        nc.tensor.matmul(
                            out,
                            whcW[:, b * 64:(b + 1) * 64],
                            tile[:, sft:sft + n],
                            start=st, stop=sp,
                            skip_group_check=True)
                    sl = slice((k % 2) * CH, (k % 2) * CH + n)
                    wsl = slice(w0, w0 + n)
                    # DVE can read only one PSUM input per op: copy col1's
                    # partial to SBUF (on ACT — DVE is the busier engine),
                    # then add col0's on DVE
                    nc.scalar.copy(g_osum[:, sl], p_ct[64:128, :n])
                    nc.vector.tensor_add(g_osum[:, sl], g_osum[:, sl],
                                         p_ct[0:64, :n])
                    nc.scalar.activation(g_osb[:, sl], g_osum[:, sl], AF.Sigmoid,
                                         bias=wsb[f"bobc{l}"][:])
                    nc.scalar.activation(g_th[:, sl], nxt_c[0:64, wsl], AF.Tanh)
                    nc.vector.tensor_mul(nxt_x[64:128, wsl], g_osb[:, sl],
                                         g_th[:, sl])
                    ring_zero(nxt_x, 64, w0, n)

                for k in range(len(CHUNKS)):
                    phase_a(k)
                    if k >= 2:
                        phase_b(k - 2)
                phase_b(len(CHUNKS) - 2)
                phase_b(len(CHUNKS) - 1)

                if l == 0 and n_layers > 1:
                    nc.sync.dma_start(hscr[t, :, :], nxt_x[64:128, :])
                if l == n_layers - 1:
                    nc.sync.dma_start(hs[t, :, :], nxt_x[64:128, :])
                    nc.sync.dma_start(cs[t, :, :], nxt_c[0:64, :])
                step += 1
    nc.compile()
    return nc


def _pack_weights(l, wxi, whi, wci, wxf, whf, wcf, wxc, whc, wxo, who, wco,
                  b_i, b_f, b_c, b_o):
    def tap(wa, wb, dy, dx):
        # [64(k), 128(m)] block: k=c_in, m = gateA|gateB c_out
        return np.concatenate([wa[l, :, :, dy, dx].T, wb[l, :, :, dy, dx].T], axis=1)

    aif = np.concatenate(
        [np.concatenate([tap(wxi, wxf, dy, dx), tap(whi, whf, dy, dx)], axis=0)
         for dy in range(3) for dx in range(3)], axis=1)

    def peep_block(wa, wb, kind, dy, dx):
        # rows 0:64 tap (dy,dx); rows 64:128 tap shifted +1 col (cc) or
        # +1 row (cc98); the (2,2) singleton gets zeroed bottom rows
        top = tap(wa, wb, dy, dx) if wb is not None else wa[l, :, :, dy, dx].T
        if (dy, dx) == (2, 2):
            bot = np.zeros_like(top)
        elif kind == 0:
            bot = tap(wa, wb, dy, dx + 1) if wb is not None else wa[l, :, :, dy, dx + 1].T
        else:
            bot = tap(wa, wb, dy + 1, dx) if wb is not None else wa[l, :, :, dy + 1, dx].T
        return np.concatenate([top, bot], axis=0)

    cif = np.concatenate(
        [peep_block(wci, wcf, kind, dy, dx)
         for kind, (dy, dx) in PEEP_BLOCKS], axis=1)

    # acto: M-halves [o | ct] so phase B's whc conv accumulates onto the
    # o-half (psum partitions 0:64)
    acto = np.concatenate(
        [np.concatenate([tap(wxo, wxc, dy, dx), tap(who, whc, dy, dx)], axis=0)
         for dy in range(3) for dx in range(3)], axis=1)

    whcT = np.concatenate(
        [peep_block(whc, None, kind, dy, dx)
         for kind, (dy, dx) in PEEP_BLOCKS], axis=1)

    return {
        f"aif{l}": np.ascontiguousarray(aif, np.float16),
        f"cif{l}": np.ascontiguousarray(cif, np.float16),
        f"acto{l}": np.ascontiguousarray(acto, np.float16),
        f"whc{l}": np.ascontiguousarray(whcT, np.float16),
        f"bif{l}": np.concatenate([b_i[l], b_f[l]]).reshape(128, 1).astype(np.float32),
        f"bc{l}": b_c[l].reshape(64, 1).astype(np.float32),
        f"bobc{l}": (b_o[l] + b_c[l]).reshape(64, 1).astype(np.float32),
    }


def kernel(x, wxi, whi, wci, wxf, whf, wcf, wxc, whc, wxo, who, wco,
           b_i, b_f, b_c, b_o):
    global LAST_EXEC_NS, LAST_RES
    t_steps, bsz = x.shape[0], x.shape[1]
    assert (t_steps, bsz) == (T, B)

    wmaps = {}
    for l in range(L):
        wmaps.update(_pack_weights(l, wxi, whi, wci, wxf, whf, wcf, wxc, whc,
                                   wxo, who, wco, b_i, b_f, b_c, b_o))

    # pad x per batch item: [B, T, C, ALLOC]
    xp = np.zeros((B, T, C, ALLOC), np.float16)
    xview = xp[:, :, :, :NPIX].reshape(B, T, C, Hp, Wp)
    xview[:, :, :, 1:97, 1:97] = np.transpose(x, (1, 0, 2, 3, 4))

    nc = _build(T, L)
    in_maps = [dict(wmaps, xp=np.ascontiguousarray(xp[b])) for b in range(B)]
    res = run_bass_kernel_spmd(nc, in_maps, core_ids=list(range(B)))
    LAST_RES = res
    LAST_EXEC_NS = res.exec_time_ns

    hs = np.zeros((T, B, C, H, W), np.float32)
    cs = np.zeros((T, B, C, H, W), np.float32)
    for b in range(B):
        hp = res.results[b]["hs"][:, :, :NPIX].astype(np.float32).reshape(T, C, Hp, Wp)
        cp = res.results[b]["cs"][:, :, :NPIX].astype(np.float32).reshape(T, C, Hp, Wp)
        hs[:, b] = hp[:, :, 1:97, 1:97]
        cs[:, b] = cp[:, :, 1:97, 1:97]
    return np.stack([hs, cs])


# revision 34
# speedup vs baseline: 1.2373x; 1.2373x over previous
"""Peephole Conv-LSTM (T=8,B=8,C=64,H=W=96,L=2,K=3) on 8 Trainium2 cores.

Strategy
--------
Data-parallel over batch: core b processes batch item b through the whole
T x L recurrence (no collectives).

Conv-as-matmul: images live in SBUF as [C=64 partitions, 98*98 flat padded
pixels].  A 3x3 SAME conv is 9 shifted matmuls accumulating in PSUM, where
the shift is just a flat AP offset into the padded image.

Partition packing (K=128 contraction, M=128 outputs per matmul):
  * xh combo  [x_pad | h_pad]      -> conv(x,Wx*) + conv(h,Wh*) fuse into one
    K=128 matmul per tap; M packs two gates per pass.
  * cc combo  [c | c<<1]  and  cc98 combo [c | c<<98] -> peephole convs pair
    taps (dy,0)+(dy,1) and (0,2)+(1,2) in K; only (2,2) is a singleton with
    zeroed bottom rows.  5 matmuls per 9-tap peephole conv instead of 9.

Per (layer, t): phase A computes i,f (psum [i|f]) and o_partial,ct
(psum [o|ct]) over 19 flat pixel chunks of <=496; c_new = ct*i + c*f written
into ping-pong cc/cc98 buffers + dup-shift copies.  Phase B (pipelined TWO
chunks behind A so the dup DMA latency hides under A's matmuls) accumulates
conv(c_new,Whc) directly onto the o-half of the SAME psum bank, then
sigmoid/tanh/mul produce h_new.  The reference's quirk (o-gate reuses
Whc + bc on the new cell state) is baked into phase B weights/biases.

x inputs double-buffer through the xh ping-pong pair: the load for step
s+1 is issued at the start of step s into the tile A(s) is not reading.
"""

import os

import numpy as np

import concourse.bass as bass
import concourse.mybir as mybir
from concourse import bacc
from concourse.bass_utils import run_bass_kernel_spmd
from concourse.tile import TileContext

F32 = mybir.dt.float32
F16 = mybir.dt.float16
AF = mybir.ActivationFunctionType

T, B, C, H, W, L, KS = 8, 8, 64, 96, 96, 2, 3
Hp = Wp = 98
NPIX = Hp * Wp              # 9604
ALLOC = NPIX + 1            # slack elem so shifted dup reads stay in bounds
W0 = Wp + 1                 # flat index of output pixel (0,0) in padded coords
NWIN = 96 * Wp + 96 - W0 + 1  # 9406 flat positions spanning all output pixels
CH = 496
CHUNKS = [(W0 + i * CH, min(CH, NWIN - i * CH)) for i in range((NWIN + CH - 1) // CH)]

# peephole 5-block structure: (tile_kind, tap_of_rows_0_64)
#   tile_kind 0 -> cc   [c | c<<1]
#   tile_kind 1 -> cc98 [c | c<<98]
# taps as (dy,dx); row-half-1 tap = half-0 tap shifted +1 (cc) or +98 (cc98)
PEEP_BLOCKS = [(0, (0, 0)), (0, (1, 0)), (0, (2, 0)), (1, (0, 2)), (0, (2, 2))]


def _tap_shift(dy, dx):
    return (dy - 1) * Wp + (dx - 1)


# profiling side-channel for test.py
LAST_EXEC_NS = None
LAST_RES = None


def _build(t_steps: int, n_layers: int, reps: int = 1) -> bass.Bass:
    nc = bacc.Bacc("TRN2", target_bir_lowering=False, debug=False,
                   enable_asserts=False, num_devices=8)

    n_steps = t_steps * n_layers
    xp = nc.declare_dram_parameter("xp", [t_steps, C, ALLOC], F16, isOutput=False)
    wd = {}
    for l in range(n_layers):
        wd[f"aif{l}"] = nc.declare_dram_parameter(f"aif{l}", [128, 9 * 128], F16, isOutput=False)
        wd[f"cif{l}"] = nc.declare_dram_parameter(f"cif{l}", [128, 5 * 128], F16, isOutput=False)
        wd[f"acto{l}"] = nc.declare_dram_parameter(f"acto{l}", [128, 9 * 128], F16, isOutput=False)
        wd[f"whc{l}"] = nc.declare_dram_parameter(f"whc{l}", [128, 5 * 64], F16, isOutput=False)
        wd[f"bif{l}"] = nc.declare_dram_parameter(f"bif{l}", [128, 1], F32, isOutput=False)
        wd[f"bc{l}"] = nc.declare_dram_parameter(f"bc{l}", [64, 1], F32, isOutput=False)
        wd[f"bobc{l}"] = nc.declare_dram_parameter(f"bobc{l}", [64, 1], F32, isOutput=False)
    hs = nc.declare_dram_parameter("hs", [t_steps, C, ALLOC], F16, isOutput=True)
    cs = nc.declare_dram_parameter("cs", [t_steps, C, ALLOC], F16, isOutput=True)
    hscr = nc.dram_tensor("hscr", [t_steps, C, ALLOC], F16) if n_layers > 1 else None

    with TileContext(nc) as tc:
        with (
            tc.tile_pool(name="big", bufs=1) as bigp,
            tc.tile_pool(name="wp", bufs=1) as wp,
            tc.tile_pool(name="gp", bufs=1) as gp,
            tc.tile_pool(name="pif", bufs=3, space="PSUM") as pool_pif,
            tc.tile_pool(name="pcto", bufs=5, space="PSUM") as pool_pcto,
        ):
            xh = [bigp.tile([128, ALLOC], F16, name="xh0"),
                  bigp.tile([128, ALLOC], F16, name="xh1")]
            cc = [bigp.tile([128, ALLOC], F16, name="cc0"),
                  bigp.tile([128, ALLOC], F16, name="cc1")]
            cc98 = [bigp.tile([128, ALLOC], F16, name="cc98_0"),
                    bigp.tile([128, ALLOC], F16, name="cc98_1")]

            # x0 load first (before weights/memsets) so phase A of step 0
            # isn't queued behind them; striped so chunk 0 starts early.
            # The x half of xh[0] is fully overwritten by this load, so the
            # zero-init below only touches the h halves.
            XQ = ALLOC // 8
            for q in range(8):
                hi = ALLOC if q == 7 else (q + 1) * XQ
                nc.sync.dma_start(xh[0][0:64, q * XQ:hi], xp[0, :, q * XQ:hi])

            wsb = {}
            for l in range(n_layers):
                for nm, cols in (("aif", 9 * 128), ("cif", 5 * 128),
                                 ("acto", 9 * 128), ("whc", 5 * 64)):
                    tl = wp.tile([128, cols], F16, name=f"{nm}sb{l}")
                    if l == 0 and nm in ("aif", "acto"):
                        # per-tap stripes: step 0's first matmuls start after
                        # one 32KB stripe instead of the full 294KB block
                        for j in range(9):
                            nc.sync.dma_start(tl[:, j * 128:(j + 1) * 128],
                                              wd[f"{nm}{l}"][:, j * 128:(j + 1) * 128])
                    else:
                        nc.sync.dma_start(tl[:], wd[f"{nm}{l}"][:])
                    wsb[f"{nm}{l}"] = tl
                for nm, p in (("bif", 128), ("bc", 64), ("bobc", 64)):
                    tl = wp.tile([p, 1], F32, name=f"{nm}sb{l}")
                    nc.sync.dma_start(tl[:], wd[f"{nm}{l}"][:])
                    wsb[f"{nm}{l}"] = tl

            # Init-zero ONLY the leading/trailing edge columns: every interior
            # byte of these tiles is (ring-zeroed and) written before it is
            # read, so full-tile memsets would just stall the Vector FIFO for
            # ~20us at step 0.  Edges per tile cover the union of what the
            # plain half and its shifted dup half can read uninitialized.
            for tl, lo, hi in ((xh[0], W0, NPIX - 99), (xh[1], W0, NPIX - 99),
                               (cc[0], W0, NPIX - 100), (cc[1], W0, NPIX - 100),
                               (cc98[0], W0, NWIN - 1), (cc98[1], W0, NWIN - 1)):
                nc.vector.memset(tl[:, 0:lo], 0.0)
                nc.vector.memset(tl[:, hi:ALLOC], 0.0)

            # PE warmup burst: ~20 dependency-light matmuls so the HAM clock
            # gate reaches 8/8 before step 0's real matmuls (results unused)
            for wi in range(20):
                p_w = pool_pif.tile([128, CH], F32, tag="pif")
                nc.tensor.matmul(p_w[:, :CH],
                                 wsb["aif0"][:, 0:128],
                                 wsb["aif0"][:, 128:128 + CH],
                                 start=True, stop=True)

            g_if = gp.tile([128, 2 * CH], F16, name="g_if")
            g_osum = gp.tile([64, 2 * CH], F32, name="g_osum")
            g_ct = gp.tile([64, 2 * CH], F16, name="g_ct")
            g_tmp = gp.tile([64, 2 * CH], F16, name="g_tmp")
            g_th = gp.tile([64, 2 * CH], F16, name="g_th")
            g_osb = gp.tile([64, 2 * CH], F16, name="g_osb")

            def src_of(step):
                l, t = divmod(step, t_steps)
                return (xp if l == 0 else hscr), t

            def ring_zero(tile, p0, w0, n):
                # zero pad-ring cols 0/97 garbage inside flat [w0, w0+n);
                # on GpSimd — it's otherwise idle and this keeps DVE free
                for rem in (0, Wp - 1):
                    first = ((w0 - rem + Wp - 1) // Wp) * Wp + rem
                    if first >= w0 + n:
                        continue
                    cnt = (w0 + n - 1 - first) // Wp + 1
                    v = tile[p0:p0 + 64, first:first + Wp * cnt]
                    v = v.rearrange("p (r w) -> p r w", w=Wp)
                    nc.gpsimd.memset(v[:, :, 0:1], 0.0)

            step = 0
            for _rep in range(reps):
             for l in range(n_layers):
              for t in range(t_steps):
                s = step
                cur_x, nxt_x = xh[s % 2], xh[(s + 1) % 2]
                cur_c, nxt_c = cc[s % 2], cc[(s + 1) % 2]
                cur_c98, nxt_c98 = cc98[s % 2], cc98[(s + 1) % 2]
                aifW, cifW = wsb[f"aif{l}"], wsb[f"cif{l}"]
                actoW, whcW = wsb[f"acto{l}"], wsb[f"whc{l}"]

                first = (t == 0)  # h_0 = c_0 = 0: K=64 matmuls on the x half
                # only, no peephole matmuls, no c*f term — so no state resets
                # are needed at the layer boundary.

                # prefetch next step's x into the other xh buffer
                if s + 1 < n_steps:
                    srcn, tn = src_of(s + 1)
                    nc.sync.dma_start(nxt_x[0:64, :], srcn[tn, :, :])

                pcto_tiles = {}

                def phase_a(k, cur_x=cur_x, cur_c=cur_c, cur_c98=cur_c98,
                            nxt_c=nxt_c, nxt_c98=nxt_c98, aifW=aifW,
                            cifW=cifW, actoW=actoW, l=l, first=first,
                            pcto_tiles=pcto_tiles):
                    w0, n = CHUNKS[k]
                    kp = 64 if first else 128  # t=0: contract x half only
                    p_if = pool_pif.tile([128, CH], F32, tag="pif")
                    p_ct = pool_pcto.tile([128, CH], F32, tag="pcto")
                    pcto_tiles[k] = p_ct
                    for j in range(9):
                        dy, dx = divmod(j, 3)
                        sft = w0 + _tap_shift(dy, dx)
                        nc.tensor.matmul(
                            p_if[:, :n],
                            aifW[0:kp, j * 128:(j + 1) * 128],
                            cur_x[0:kp, sft:sft + n],
                            start=(j == 0), stop=(first and j == 8))
                    if not first:
                        for b, (kind, (dy, dx)) in enumerate(PEEP_BLOCKS):
                            tile = cur_c if kind == 0 else cur_c98
                            sft = w0 + _tap_shift(dy, dx)
                            nc.tensor.matmul(
                                p_if[:, :n],
                                cifW[:, b * 128:(b + 1) * 128],
                                tile[:, sft:sft + n],
                                start=False, stop=(b == 4))
                    for j in range(9):
                        dy, dx = divmod(j, 3)
                        sft = w0 + _tap_shift(dy, dx)
                        # stop=True closes the sim's accumulation-group so the
                        # ct half can be read now; phase B reopens with
                        # skip_group_check and accumulates onto the o half.
                        nc.tensor.matmul(
                            p_ct[:, :n],
                            actoW[0:kp, j * 128:(j + 1) * 128],
                            cur_x[0:kp, sft:sft + n],
                            start=(j == 0), stop=(j == 8))

                    sl = slice((k % 2) * CH, (k % 2) * CH + n)
                    wsl = slice(w0, w0 + n)
                    nc.scalar.activation(g_if[:, sl], p_if[:, :n], AF.Sigmoid,
                                         bias=wsb[f"bif{l}"][:])
                    nc.scalar.activation(g_ct[:, sl], p_ct[64:128, :n], AF.Tanh,
                                         bias=wsb[f"bc{l}"][:])
                    if first:
                        # c_new = ct*i
                        nc.vector.tensor_mul(nxt_c[0:64, wsl], g_ct[:, sl],
                                             g_if[0:64, sl])
                    else:
                        # c_new = c*f + ct*i ; read c via the +1-shift dup half
                        # so both TensorTensor inputs share start partition 64
                        nc.vector.tensor_mul(nxt_c[0:64, wsl],
                                             cur_c[64:128, w0 - 1:w0 - 1 + n],
                                             g_if[64:128, sl])
                        nc.vector.tensor_mul(g_tmp[:, sl], g_ct[:, sl],
                                             g_if[0:64, sl])
                        nc.vector.tensor_add(nxt_c[0:64, wsl],
                                             nxt_c[0:64, wsl],
                                             g_tmp[:, sl])
                    # ring-zero this chunk of c_new, then its dup stripes
                    ring_zero(nxt_c, 0, w0, n)
                    nc.sync.dma_start(nxt_c[64:128, w0 - 1:w0 - 1 + n],
                                      nxt_c[0:64, w0:w0 + n])
                    nc.sync.dma_start(nxt_c98[0:64, w0:w0 + n],
                                      nxt_c[0:64, w0:w0 + n])
                    nc.sync.dma_start(nxt_c98[64:128, w0 - 98:w0 - 98 + n],
                                      nxt_c[0:64, w0:w0 + n])

                def phase_b(k, nxt_x=nxt_x, nxt_c=nxt_c, nxt_c98=nxt_c98,
                            whcW=whcW, l=l, pcto_tiles=pcto_tiles):
                    w0, n = CHUNKS[k]
                    p_ct = pcto_tiles[k]
                    # col-split: blocks 0-2 accumulate onto the o half
                    # (partitions 0:64, col group 0); blocks 3-4 overwrite the
                    # consumed ct half (col group 1) and run concurrently on
                    # the other half of the PE array.  Interleave issue order
                    # so the hardware overlaps adjacent different-col MMs.
                    order = [0, 3, 1, 4, 2]
                    for b in order:
                        kind, (dy, dx) = PEEP_BLOCKS[b]
                        tile = nxt_c if kind == 0 else nxt_c98
                        sft = w0 + _tap_shift(dy, dx)
                        if b < 3:
                            out, st, sp = p_ct[0:64, :n], False, (b == 2)
                        else:
                            out, st, sp = p_ct[64:128, :n], (b == 3), (b == 4)
                        nc.tensor.matmul(
                            out,
                            whcW[:, b * 64:(b + 1) * 64],
                            tile[:, sft:sft + n],
                            start=st, stop=sp,
                            skip_group_check=True)
                    sl = slice((k % 2) * CH, (k % 2) * CH + n)
                    wsl = slice(w0, w0 + n)
                    # DVE can read only one PSUM input per op: copy col1's
                    # partial to SBUF, then add col0's
                    nc.vector.tensor_copy(g_osum[:, sl], p_ct[64:128, :n])
                    nc.vector.tensor_add(g_osum[:, sl], g_osum[:, sl],
                                         p_ct[0:64, :n])
                    nc.scalar.activation(g_osb[:, sl], g_osum[:, sl], AF.Sigmoid,
                                         bias=wsb[f"bobc{l}"][:])
                    nc.scalar.activation(g_th[:, sl], nxt_c[0:64, wsl], AF.Tanh)
                    nc.vector.tensor_mul(nxt_x[64:128, wsl], g_osb[:, sl],
                                         g_th[:, sl])
                    ring_zero(nxt_x, 64, w0, n)

                for k in range(len(CHUNKS)):
                    phase_a(k)
                    if k >= 3:
                        phase_b(k - 3)
                phase_b(len(CHUNKS) - 3)
                phase_b(len(CHUNKS) - 2)
                phase_b(len(CHUNKS) - 1)

                if l == 0 and n_layers > 1:
                    nc.sync.dma_start(hscr[t, :, :], nxt_x[64:128, :])
                if l == n_layers - 1:
                    nc.sync.dma_start(hs[t, :, :], nxt_x[64:128, :])
                    nc.sync.dma_start(cs[t, :, :], nxt_c[0:64, :])
                step += 1
    nc.compile()
    return nc


def _pack_weights(l, wxi, whi, wci, wxf, whf, wcf, wxc, whc, wxo, who, wco,
                  b_i, b_f, b_c, b_o):
    def tap(wa, wb, dy, dx):
        # [64(k), 128(m)] block: k=c_in, m = gateA|gateB c_out
        return np.concatenate([wa[l, :, :, dy, dx].T, wb[l, :, :, dy, dx].T], axis=1)

    aif = np.concatenate(
        [np.concatenate([tap(wxi, wxf, dy, dx), tap(whi, whf, dy, dx)], axis=0)
         for dy in range(3) for dx in range(3)], axis=1)

    def peep_block(wa, wb, kind, dy, dx):
        # rows 0:64 tap (dy,dx); rows 64:128 tap shifted +1 col (cc) or
        # +1 row (cc98); the (2,2) singleton gets zeroed bottom rows
        top = tap(wa, wb, dy, dx) if wb is not None else wa[l, :, :, dy, dx].T
        if (dy, dx) == (2, 2):
            bot = np.zeros_like(top)
        elif kind == 0:
            bot = tap(wa, wb, dy, dx + 1) if wb is not None else wa[l, :, :, dy, dx + 1].T
        else:
            bot = tap(wa, wb, dy + 1, dx) if wb is not None else wa[l, :, :, dy + 1, dx].T
        return np.concatenate([top, bot], axis=0)

    cif = np.concatenate(
        [peep_block(wci, wcf, kind, dy, dx)
         for kind, (dy, dx) in PEEP_BLOCKS], axis=1)

    # acto: M-halves [o | ct] so phase B's whc conv accumulates onto the
    # o-half (psum partitions 0:64)
    acto = np.concatenate(
        [np.concatenate([tap(wxo, wxc, dy, dx), tap(who, whc, dy, dx)], axis=0)
         for dy in range(3) for dx in range(3)], axis=1)

    whcT = np.concatenate(
        [peep_block(whc, None, kind, dy, dx)
         for kind, (dy, dx) in PEEP_BLOCKS], axis=1)

    return {
        f"aif{l}": np.ascontiguousarray(aif, np.float16),
        f"cif{l}": np.ascontiguousarray(cif, np.float16),
        f"acto{l}": np.ascontiguousarray(acto, np.float16),
        f"whc{l}": np.ascontiguousarray(whcT, np.float16),
        f"bif{l}": np.concatenate([b_i[l], b_f[l]]).reshape(128, 1).astype(np.float32),
        f"bc{l}": b_c[l].reshape(64, 1).astype(np.float32),
        f"bobc{l}": (b_o[l] + b_c[l]).reshape(64, 1).astype(np.float32),
    }


def kernel(x, wxi, whi, wci, wxf, whf, wcf, wxc, whc, wxo, who, wco,
           b_i, b_f, b_c, b_o):
    global LAST_EXEC_NS, LAST_RES
    t_steps, bsz = x.shape[0], x.shape[1]
    assert (t_steps, bsz) == (T, B)

    wmaps = {}
    for l in range(L):
        wmaps.update(_pack_weights(l, wxi, whi, wci, wxf, whf, wcf, wxc, whc,
                                   wxo, who, wco, b_i, b_f, b_c, b_o))

    # pad x per batch item: [B, T, C, ALLOC]
    xp = np.zeros((B, T, C, ALLOC), np.float16)
    xview = xp[:, :, :, :NPIX].reshape(B, T, C, Hp, Wp)
    xview[:, :, :, 1:97, 1:97] = np.transpose(x, (1, 0, 2, 3, 4))

    nc = _build(T, L)
    in_maps = [dict(wmaps, xp=np.ascontiguousarray(xp[b])) for b in range(B)]
    res = run_bass_kernel_spmd(nc, in_maps, core_ids=list(range(B)))
    LAST_RES = res
    LAST_EXEC_NS = res.exec_time_ns

    hs = np.zeros((T, B, C, H, W), np.float32)
    cs = np.zeros((T, B, C, H, W), np.float32)
    for b in range(B):
        hp = res.results[b]["hs"][:, :, :NPIX].astype(np.float32).reshape(T, C, Hp, Wp)
        cp = res.results[b]["cs"][:, :, :NPIX].astype(np.float32).reshape(T, C, Hp, Wp)
        hs[:, b] = hp[:, :, 1:97, 1:97]
        cs[:, b] = cp[:, :, 1:97, 1:97]
    return np.stack([hs, cs])
